# revision 1
# baseline (speedup 1.0000x reference)
"""Trainium2 Bass kernel for an 8-head self-attention block with relative
position embeddings (LayerNorm -> qkv -> rel-pos attention -> out proj).

Sharding: pure data-parallel over the batch dim. B == 8 == n_cores, so each
NeuronCore processes one batch element end-to-end; no collectives.

Math notes (per batch element, per head h):
  scores = ((q+u)@k^T + (q+v)@pos^T) / 8
         = ((q+u) @ (k+pos)^T + (v-u).pos[m]) / 8
Host precomputes pos = rel_pos_emb @ w_pos^T (input-dependent only through
the weight w_pos), so on device:
  - q' = q + u_h folds into the q PSUM->SBUF copy (per-partition add),
  - kp = k + pos folds into the k PSUM->SBUF copy (tensor_tensor add),
  - the remaining bias c_h[m] = (v_h-u_h).pos_h[m] is a host-precomputed
    DRAM table that the ACT engine folds into the softmax exp:
      P^T = Exp(S_psum * 0.125 + c_col).
Softmax denominators come from a ones-column appended to V (M=65 in the
P^T @ V matmul); the division is one reciprocal_approx_fast custom-DVE op
(SBUF-source only - PSUM reads misparse) + a GPSIMD partition-broadcast +
one DVE multiply. The out projection packs head pairs along K (K=128 per
matmul instead of 65) with b_out as a K=1 ones-row matmul, accumulating the
last pair's contribution after the others so it overlaps that pair's
softmax divisions.

Everything on the matmul path runs bf16 (weights, z, q', kp, P^T, V, out
proj); LayerNorm, PSUM accumulation and the softmax bias/exp stay f32.
(fp8e4 P/V with DoubleRow was tried and is ~25us faster on PE but pushes
the end-to-end error to ~3e-2, over the 2e-2 gate.)

Schedule: the per-pair scores loop is ACT-paced (2 exps of [128,1024] per
mt iteration vs 4 small score matmuls), so one contiguous "filler" chain -
a v-projection chain for pair 0, a previous pair's PV chain otherwise - is
emitted per mt iteration to keep the PE dense while ACT chews. Fillers must
be whole accumulation chains: interleaving individual chain steps between
other matmul groups measurably slows the PE stream. The PT ring holds two
pairs (bufs=4) since pair t's exps write while pair t-1's woven PV chains
still read. Score matmuls of the two heads in a pair alternate (64,128)
stationary quadrants (rows 0-63 / 64-127) so their LDWEIGHTS ping-pong.
"""

import math

import numpy as np

MODEL_DIM = 512
NUM_HEADS = 8
DIM_HEAD = 64
INNER = NUM_HEADS * DIM_HEAD
B, L = 8, 1024
EPS = 1e-5
N_CORES = 8
LT = L // 128          # l tiles
KT = MODEL_DIM // 128  # contraction (d) tiles
SCALE = DIM_HEAD ** -0.5

_CACHE = {}


def _rel_pos_emb_np():
    # mirror reference._rel_pos_emb in float32 numpy
    rel = (np.arange(L, dtype=np.float32)[:, None]
           - np.arange(MODEL_DIM, dtype=np.float32)[None, :])
    freqs = np.exp(-np.arange(0, 2 * MODEL_DIM, 2, dtype=np.float32)
                   * np.float32(math.log(10000.0) / MODEL_DIM))
    angle = rel * freqs[None, :]
    even = (np.arange(MODEL_DIM) % 2) == 0
    return np.where(even[None, :], np.cos(angle), np.sin(angle)).astype(np.float32)


def _build():
    import concourse.bacc as bacc
    import concourse.tile as tile
    from concourse import mybir

    F32 = mybir.dt.float32
    F32R = mybir.dt.float32r
    BF16 = mybir.dt.bfloat16
    FP8 = mybir.dt.float8e4
    AF = mybir.ActivationFunctionType
    ALU = mybir.AluOpType

    nc = bacc.Bacc('TRN2', target_bir_lowering=False)

    xb_d = nc.dram_tensor('xb', [L, MODEL_DIM], F32, kind='ExternalInput')
    w1qk_d = nc.dram_tensor('w1qk', [MODEL_DIM, 2 * INNER], BF16, kind='ExternalInput')
    w1v_d = nc.dram_tensor('w1v', [MODEL_DIM, INNER], BF16, kind='ExternalInput')
    posT_d = nc.dram_tensor('posT', [MODEL_DIM, L], BF16, kind='ExternalInput')
    cb_d = nc.dram_tensor('cb', [128, LT, NUM_HEADS], F32, kind='ExternalInput')
    ucol_d = nc.dram_tensor('ucol', [128, KT], F32, kind='ExternalInput')
    wout2b_d = nc.dram_tensor('wout2b', [128, KT, MODEL_DIM], BF16, kind='ExternalInput')
    bout_d = nc.dram_tensor('bout', [1, MODEL_DIM], BF16, kind='ExternalInput')
    onesr_d = nc.dram_tensor('onesr', [1, 128], BF16, kind='ExternalInput')
    ident_d = nc.dram_tensor('ident', [128, 128], BF16, kind='ExternalInput')
    ob_d = nc.dram_tensor('ob', [L, MODEL_DIM], F32, kind='ExternalOutput')

    with nc.allow_low_precision(reason="bf16 matmul pipeline"), \
            tile.TileContext(nc) as tc:
        with (
            tc.tile_pool(name='const', bufs=1) as constp,
            tc.tile_pool(name='acts', bufs=1) as acts,
            tc.tile_pool(name='wts', bufs=1) as wts,
            tc.tile_pool(name='xz', bufs=4) as xzp,
            tc.tile_pool(name='xtp', bufs=6) as xtp,
            tc.tile_pool(name='qkp', bufs=2) as qkp,
            tc.tile_pool(name='pt', bufs=4) as ptp,
            tc.tile_pool(name='rc', bufs=4) as rcp,
            tc.tile_pool(name='outp', bufs=3) as outp,
            tc.tile_pool(name='psM', bufs=2, space='PSUM') as psM,
        ):
            # ---------------- constants / weights ----------------
            cb_sb = constp.tile([128, LT, NUM_HEADS], F32)
            ucol_sb = constp.tile([128, KT], F32)
            ident_sb = constp.tile([128, 128], BF16)
            nc.gpsimd.dma_start(ident_sb[:], ident_d[:])
            eps_sb = constp.tile([128, 1], F32)
            nc.vector.memset(eps_sb[:], EPS)

            # startup is DMA-bandwidth-bound: the sync queue carries ONLY the
            # x tiles (the LN critical path); w1qk dispatches sit mid-LN in the
            # scalar queue; everything else is gated behind the last x arrival
            # (see the dummy gpsimd copy in the LN loop below)
            w1v_sb = wts.tile([128, KT, INNER], BF16)
            w1v_r = w1v_d[:].rearrange('(t p) r -> p t r', p=128)
            w1qk_sb = wts.tile([128, KT, 2 * INNER], BF16)
            w1qk_r = w1qk_d[:].rearrange('(t p) r -> p t r', p=128)
            posT_sb = wts.tile([128, KT, L], BF16)
            posT_r = posT_d[:].rearrange('(t p) m -> p t m', p=128)
            wout2b_sb = wts.tile([128, KT, MODEL_DIM], BF16)
            bout_sb = constp.tile([1, MODEL_DIM], BF16)
            onesr_sb = constp.tile([1, 128], BF16)
            gate_sb = constp.tile([1, 1], F32)

            zT = acts.tile([128, KT, L], BF16)
            # V with a ones column per head: [m-part, mt, h*65+c]
            v_sb = acts.tile([128, LT, NUM_HEADS * 65], BF16)
            nc.vector.memset(
                v_sb[:].rearrange('p t (h c) -> p t h c', c=65)[:, :, :, 64:65], 1.0)
            # normalized attention output, head pairs packed: [128, pair, L]
            outT2 = acts.tile([128, KT, L], BF16)

            # ---------------- LayerNorm + transpose into zT ----------------
            # software-pipelined by one stage: stats/aggr of tile lt are
            # emitted before the normalize/transpose of tile lt-1 so the
            # in-order DVE queue reaches zt(0) as soon as its rstd is ready
            ln_state = {}

            def ln_front(lt):
                xt = xtp.tile([128, MODEL_DIM], F32, tag='xt', name=f'xt{lt}')
                # x alternates two queues: a single DMA ring sustains only
                # ~128GB/s, which throttled the LN pipeline; x0 is further
                # split across two rings (its landing gates everything)
                if lt == 0:
                    nc.sync.dma_start(xt[0:64, :], xb_d[0:64, :])
                    nc.scalar.dma_start(xt[64:128, :], xb_d[64:128, :])
                else:
                    eng = nc.sync if lt % 2 == 0 else nc.gpsimd
                    eng.dma_start(xt[:], xb_d[lt * 128:(lt + 1) * 128, :])
                stats = xzp.tile([128, 6], F32, tag='stats')
                nc.vector.bn_stats(stats[:], xt[:])
                mv = xzp.tile([128, 2], F32, tag='mv')
                nc.vector.bn_aggr(mv[:], stats[:])
                lnv = xzp.tile([128, 1], F32, tag='lnv')
                nc.scalar.activation(lnv[:], mv[:, 1:2], AF.Ln, bias=eps_sb[:], scale=1.0)
                rstd = xzp.tile([128, 1], F32, tag='rstd', name=f'rstd{lt}')
                nc.scalar.activation(rstd[:], lnv[:], AF.Exp, scale=-0.5)
                ln_state[lt] = (xt, mv, rstd)
                if lt == LT - 1:
                    # gate: runs once the last x tile has landed, keeping the
                    # deferred weight DMAs off the startup-critical bandwidth
                    nc.gpsimd.tensor_copy(gate_sb[:], xt[0:1, 0:1])
                    nc.gpsimd.dma_start(ucol_sb[:], ucol_d[:])
                    nc.gpsimd.dma_start(cb_sb[:], cb_d[:])
                    nc.gpsimd.dma_start(w1v_sb[:], w1v_r)
                    for kt in range(KT):
                        nc.gpsimd.dma_start(posT_sb[:, kt, :], posT_r[:, kt, :])
                    nc.gpsimd.dma_start(wout2b_sb[:], wout2b_d[:])
                    nc.gpsimd.dma_start(bout_sb[:], bout_d[:])
                    nc.gpsimd.dma_start(onesr_sb[:], onesr_d[:])

            def ln_back(lt):
                xt, mv, rstd = ln_state.pop(lt)
                nmr = xzp.tile([128, 1], F32, tag='nmr')
                nc.vector.scalar_tensor_tensor(nmr[:], mv[:, 0:1], -1.0, rstd[:],
                                               op0=ALU.mult, op1=ALU.mult)
                zt = xzp.tile([128, MODEL_DIM], BF16, tag='zt')
                nc.vector.tensor_scalar(zt[:], xt[:], rstd[:], nmr[:],
                                        op0=ALU.mult, op1=ALU.add)
                tp = psM.tile([128, L], F32, tag='S')
                tpb = tp.bitcast(BF16)
                for c in range(KT):
                    nc.tensor.transpose(tpb[:, c * 128:(c + 1) * 128],
                                        zt[:, c * 128:(c + 1) * 128], ident_sb[:])
                nc.vector.tensor_copy(
                    zT[:, :, lt * 128:(lt + 1) * 128],
                    tpb[:, 0:512].rearrange('p (c l) -> p c l', c=KT))
                if lt == 2:
                    for kt in range(KT):
                        nc.scalar.dma_start(w1qk_sb[:, kt, :], w1qk_r[:, kt, :])

            for lt in range(LT):
                ln_front(lt)
                if lt >= 1:
                    ln_back(lt - 1)
            ln_back(LT - 1)


            # ---------------- interleaved projections + attention ----------
            qts, kps = {}, {}

            def qk_chains(t):
                """allocate qt/kp for pair t and return the 4 projection
                chains as closures (woven into the previous pair's scores
                loop as PE fillers; they depend only on zT and w1qk)"""
                qt = qkp.tile([128, L], BF16, tag='qT', name=f'qT{t}')
                kp = qkp.tile([128, L], BF16, tag='kT', name=f'kT{t}')
                qts[t], kps[t] = qt, kp
                chains = []
                for which, dst in ((1, kp), (0, qt)):
                    for lc in range(2):
                        def chain(which=which, dst=dst, lc=lc):
                            rt = which * 4 + t
                            ls = slice(lc * 512, (lc + 1) * 512)
                            acc = psM.tile([128, 512], F32, tag='qkv',
                                           name=f'qk{t}_{which}_{lc}')
                            for kt in range(KT):
                                nc.tensor.matmul(
                                    acc[:], w1qk_sb[:, kt, rt * 128:(rt + 1) * 128],
                                    zT[:, kt, ls],
                                    start=(kt == 0), stop=(kt == KT - 1))
                            if which == 0:
                                # q' = q + u_h  (per-partition add)
                                nc.vector.tensor_scalar_add(dst[:, ls], acc[:],
                                                            ucol_sb[:, t:t + 1])
                            else:
                                # kp = k + pos
                                nc.vector.tensor_tensor(dst[:, ls], acc[:],
                                                        posT_sb[:, t, ls], op=ALU.add)
                        chains.append(chain)
                return chains

            def emit_scores(t, fillers=()):
                """Scores + exp for heads 2t, 2t+1, interleaved at mt grain so
                the two heads' (64,128) stationary tiles ping-pong quadrants.
                The scores loop is ACT-paced (2 exps per mt vs 4 small score
                matmuls), so one contiguous filler chain (a v-projection or a
                previous pair's PV chain) is emitted per mt iteration to keep
                the PE dense while ACT chews."""
                qt, kp = qts[t], kps[t]
                fillers = list(fillers)
                PTs = {}
                for h in (2 * t, 2 * t + 1):
                    PTs[h] = ptp.tile([128, LT, L], BF16, tag='PT', name=f'PT{h}')
                for mt in range(LT):
                    for h in (2 * t, 2 * t + 1):
                        hp = 64 * (h % 2)
                        prow = slice(hp, hp + 64)
                        sacc = psM.tile([128, L], F32, tag='S')
                        for lc in range(2):
                            ls = slice(lc * 512, (lc + 1) * 512)
                            nc.tensor.matmul(sacc[:, ls],
                                             kp[prow, mt * 128:(mt + 1) * 128],
                                             qt[prow, ls], start=True, stop=True)
                        nc.scalar.activation(PTs[h][:, mt, :], sacc[:], AF.Exp,
                                             bias=cb_sb[:, mt, h:h + 1], scale=SCALE)
                    if fillers:
                        fillers.pop(0)()
                while fillers:
                    fillers.pop(0)()
                return PTs

            def pv_step(pvacc, h, PT, lc, mt):
                ls = slice(lc * 512, (lc + 1) * 512)
                nc.tensor.matmul(pvacc[:], v_sb[:, mt, h * 65:(h + 1) * 65],
                                 PT[:, mt, ls],
                                 start=(mt == 0), stop=(mt == LT - 1))

            def pv_finish(pvacc, h, lc):
                """normalize the finished P^T @ [V|1] into packed outT2"""
                pair, hp = divmod(h, 2)
                rows = slice(hp * 64, hp * 64 + 64)
                ls = slice(lc * 512, (lc + 1) * 512)
                zrow = rcp.tile([1, 512], F32, tag='zrow')
                nc.vector.tensor_copy(zrow[:], pvacc[64:65, :])
                rc = rcp.tile([1, 512], F32, tag='rc')
                # custom-DVE op: SBUF source only (PSUM reads misparse)
                nc.vector.reciprocal_approx_fast(rc[:], zrow[:])
                rcb = rcp.tile([64, 512], F32, tag='rcb')
                nc.gpsimd.partition_broadcast(rcb[:], rc[:])
                nc.vector.tensor_mul(outT2[rows, pair, ls], pvacc[0:64, :], rcb[:])

            def emit_pv_block(hs, PTs_, lc):
                """contiguous PV chains for the given heads at one lc"""
                for h in hs:
                    pv_chain(h, PTs_[h], lc)

            def pv_chain(h, PT, lc):
                pvacc = psM.tile([65, 512], F32, tag='pv', name=f'pv{h}_{lc}')
                for mt in range(LT):
                    pv_step(pvacc, h, PT, lc, mt)
                pv_finish(pvacc, h, lc)

            def v_chain(mt):
                acc = psM.tile([128, 512], F32, tag='qkv', name=f'vacc{mt}')
                for kt in range(KT):
                    nc.tensor.matmul(acc[:], zT[:, kt, mt * 128:(mt + 1) * 128],
                                     w1v_sb[:, kt, :],
                                     start=(kt == 0), stop=(kt == KT - 1))
                nc.vector.tensor_copy(
                    v_sb[:, mt, :].rearrange('p (h c) -> p h c', c=65)[:, :, 0:64],
                    acc[:].rearrange('p (h c) -> p h c', c=64))

            def emit_outproj(lts):
                # the last head pair's contribution is accumulated LAST so the
                # first matmuls of each chain run while that pair's softmax
                # divisions are still in flight on DVE/Pool
                for lt in lts:
                    facc = psM.tile([128, MODEL_DIM], F32, tag='qkv')
                    for p in range(KT - 1):
                        nc.tensor.matmul(facc[:], outT2[:, p, lt * 128:(lt + 1) * 128],
                                         wout2b_sb[:, p, :],
                                         start=(p == 0), stop=False)
                    nc.tensor.matmul(facc[:], onesr_sb[:], bout_sb[:],
                                     start=False, stop=False)
                    nc.tensor.matmul(facc[:], outT2[:, KT - 1, lt * 128:(lt + 1) * 128],
                                     wout2b_sb[:, KT - 1, :],
                                     start=False, stop=True)
                    ot = outp.tile([128, MODEL_DIM], F32, tag='ot')
                    nc.vector.tensor_copy(ot[:], facc[:])
                    nc.scalar.dma_start(ob_d[lt * 128:(lt + 1) * 128, :], ot[:])

            prev = None
            for c in qk_chains(0):
                c()
            for t in range(KT):
                if prev is None:
                    base = [(lambda mt=mt: v_chain(mt)) for mt in range(LT)]
                else:
                    base = [(lambda h=h, lc=lc, PT=prev[h]: pv_chain(h, PT, lc))
                            for lc in (0, 1) for h in prev.keys()]
                nxt = qk_chains(t + 1) if t + 1 < KT else []
                # interleave so the next pair's projections (and their DVE
                # folds) finish well before its scores loop begins
                fillers = []
                for i in range(max(len(base), len(nxt))):
                    if i < len(base):
                        fillers.append(base[i])
                    if i < len(nxt):
                        fillers.append(nxt[i])
                prev = emit_scores(t, fillers)
            # last pair: all PV chains; the lc0 divisions (DVE/Pool latency)
            # hide under the lc1 chains, and the lc1 divisions hide under the
            # first half of the output projection
            phs = list(prev.keys())
            emit_pv_block(phs, prev, 0)
            emit_pv_block(phs, prev, 1)
            emit_outproj(range(0, 4))
            emit_outproj(range(4, 8))

    # Force all activations (Ln/Exp/Identity) onto the single table set that
    # contains them all — otherwise the table-load picker alternates between
    # the natural_log and exp sets, paying a ~2.7us table load per switch.
    import concourse.bacc as bacc_mod
    orig_tables = bacc_mod.get_activation_tables

    def _only_ln_exp(arch):
        t = orig_tables(arch)
        return {name: (funcs if name == 'natural_log_exp_and_others' else
                       type(funcs)())
                for name, funcs in t.items()}

    bacc_mod.get_activation_tables = _only_ln_exp
    try:
        nc.compile()
    finally:
        bacc_mod.get_activation_tables = orig_tables
    return nc


def _host_prep(x, gamma, beta, w_qkv, b_qkv, w_pos, w_out, b_out, u_bias, v_bias):
    """Host-side layout prep. Returns (common_inputs, per_core_x_list)."""
    import ml_dtypes
    BF = ml_dtypes.bfloat16
    W1 = (gamma[:, None] * w_qkv.T).astype(np.float32)        # [D, 3*INNER]
    b1 = (b_qkv + beta @ w_qkv.T).astype(np.float32)
    if np.any(b1 != 0):
        raise NotImplementedError("nonzero qkv bias not supported by this kernel")
    w1qk = np.ascontiguousarray(W1[:, :2 * INNER]).astype(BF)
    w1v = np.ascontiguousarray(W1[:, 2 * INNER:]).astype(BF)
    # pos = rel_pos_emb @ w_pos^T, computed on host: [L(m), INNER]
    pos = (_rel_pos_emb_np() @ w_pos.T).astype(np.float32)
    posT = np.ascontiguousarray(pos.T).astype(BF)              # [INNER, L]
    # exp bias table c_h[m] = SCALE * (v_h - u_h) . pos_h[m], m-major tiles
    dvu = (v_bias - u_bias).astype(np.float32)                 # [H, Dh]
    cb = np.einsum('mhd,hd->mh', pos.reshape(L, NUM_HEADS, DIM_HEAD), dvu)
    cb = cb * SCALE                                            # [L(m), H]
    cb = cb.reshape(LT, 128, NUM_HEADS).transpose(1, 0, 2)
    cb = np.ascontiguousarray(cb).astype(np.float32)           # [128, LT, H]
    # u columns: ucol[:, t] = [u_{2t} | u_{2t+1}]
    ucol = np.ascontiguousarray(
        u_bias.reshape(KT, 128).T).astype(np.float32)          # [128, KT]
    # out projection, head pairs packed along K
    wout2b = np.ascontiguousarray(
        w_out.T.reshape(KT, 128, MODEL_DIM).transpose(1, 0, 2)).astype(BF)
    common = {
        'w1qk': w1qk, 'w1v': w1v, 'posT': posT, 'cb': cb, 'ucol': ucol,
        'wout2b': wout2b,
        'bout': b_out.reshape(1, MODEL_DIM).astype(BF),
        'onesr': np.ones((1, 128), BF),
        'ident': np.eye(128).astype(BF),
    }
    xs = [np.ascontiguousarray(x[b]) for b in range(N_CORES)]
    return common, xs


def kernel(x, gamma, beta, w_qkv, b_qkv, w_pos, w_out, b_out, u_bias, v_bias):
    x = np.asarray(x, np.float32)
    args = [np.asarray(a, np.float32) for a in
            (gamma, beta, w_qkv, b_qkv, w_pos, w_out, b_out, u_bias, v_bias)]
    common, xs = _host_prep(x, *args)

    if 'nc' not in _CACHE:
        _CACHE['nc'] = _build()
    nc = _CACHE['nc']

    from concourse.bass_utils import run_bass_kernel_spmd
    in_maps = [{'xb': xs[b], **common} for b in range(N_CORES)]
    res = run_bass_kernel_spmd(nc, in_maps, core_ids=list(range(N_CORES)))
    return np.stack([res.results[b]['ob'] for b in range(N_CORES)], axis=0)



# revision 28
# speedup vs baseline: 1.0213x; 1.0213x over previous
"""Trainium2 Bass kernel for an 8-head self-attention block with relative
position embeddings (LayerNorm -> qkv -> rel-pos attention -> out proj).

Sharding: pure data-parallel over the batch dim. B == 8 == n_cores, so each
NeuronCore processes one batch element end-to-end; no collectives.

Math notes (per batch element, per head h):
  scores = ((q+u)@k^T + (q+v)@pos^T) / 8
         = ((q+u) @ (k+pos)^T + (v-u).pos[m]) / 8
Host precomputes pos = rel_pos_emb @ w_pos^T (input-dependent only through
the weight w_pos), so on device:
  - q' = q + u_h folds into the q PSUM->SBUF copy (per-partition add),
  - kp = k + pos folds into the k PSUM->SBUF copy (tensor_tensor add),
  - the remaining bias c_h[m] = (v_h-u_h).pos_h[m] is a host-precomputed
    DRAM table that the ACT engine folds into the softmax exp:
      P^T = Exp(S_psum * 0.125 + c_col).
Softmax denominators come from a ones-column appended to V (M=65 in the
P^T @ V matmul); the division is one reciprocal_approx_fast custom-DVE op
(partition-0 SBUF source only - PSUM and partition-offset sources both
misparse on HW) + a GPSIMD partition-broadcast + one DVE multiply. The
last pair's lc1 PV chains borrow the S PSUM ring (idle after the final
exps) so all 4 end chains hold distinct banks and none WAR-stalls on a
previous chain's division. The out projection packs head pairs along K
(K=128 per matmul instead of 65), folds b_out into the PSUM->SBUF copy
(DVE broadcast add), and accumulates the last pair's contribution after
the others so it overlaps that pair's softmax divisions.

Everything on the matmul path runs bf16 (weights, z, q', kp, P^T, V, out
proj); LayerNorm, PSUM accumulation and the softmax bias/exp stay f32.
(fp8e4 P/V with DoubleRow was tried and is ~25us faster on PE but pushes
the end-to-end error to ~3e-2, over the 2e-2 gate.)

Schedule: the per-pair scores loop is ACT-paced (2 exps of [128,1024] per
mt iteration vs 4 small score matmuls), so one contiguous "filler" chain -
a v-projection chain for pair 0, a previous pair's PV chain otherwise - is
emitted per mt iteration to keep the PE dense while ACT chews. Fillers must
be whole accumulation chains: interleaving individual chain steps between
other matmul groups measurably slows the PE stream. The PT ring holds two
pairs (bufs=4) since pair t's exps write while pair t-1's woven PV chains
still read. Score matmuls of the two heads in a pair alternate (64,128)
stationary quadrants (rows 0-63 / 64-127) so their LDWEIGHTS ping-pong.
"""

import math

import numpy as np

MODEL_DIM = 512
NUM_HEADS = 8
DIM_HEAD = 64
INNER = NUM_HEADS * DIM_HEAD
B, L = 8, 1024
EPS = 1e-5
N_CORES = 8
LT = L // 128          # l tiles
KT = MODEL_DIM // 128  # contraction (d) tiles
SCALE = DIM_HEAD ** -0.5

_CACHE = {}


def _rel_pos_emb_np():
    # mirror reference._rel_pos_emb in float32 numpy
    rel = (np.arange(L, dtype=np.float32)[:, None]
           - np.arange(MODEL_DIM, dtype=np.float32)[None, :])
    freqs = np.exp(-np.arange(0, 2 * MODEL_DIM, 2, dtype=np.float32)
                   * np.float32(math.log(10000.0) / MODEL_DIM))
    angle = rel * freqs[None, :]
    even = (np.arange(MODEL_DIM) % 2) == 0
    return np.where(even[None, :], np.cos(angle), np.sin(angle)).astype(np.float32)


def _build(debug=False):
    import concourse.bacc as bacc
    import concourse.tile as tile
    from concourse import mybir

    F32 = mybir.dt.float32
    F32R = mybir.dt.float32r
    BF16 = mybir.dt.bfloat16
    FP8 = mybir.dt.float8e4
    AF = mybir.ActivationFunctionType
    ALU = mybir.AluOpType

    nc = bacc.Bacc('TRN2', target_bir_lowering=False)

    xb_d = nc.dram_tensor('xb', [L, MODEL_DIM], F32, kind='ExternalInput')
    # pair-major qk weights: [p, pair, kt, 0:128]=q_pair, [.,128:256]=k_pair
    w1qk_d = nc.dram_tensor('w1qk', [128, KT, KT, 256], BF16, kind='ExternalInput')
    w1v_d = nc.dram_tensor('w1v', [MODEL_DIM, INNER], BF16, kind='ExternalInput')
    posT_d = nc.dram_tensor('posT', [MODEL_DIM, L], BF16, kind='ExternalInput')
    cb_d = nc.dram_tensor('cb', [128, LT, NUM_HEADS], F32, kind='ExternalInput')
    ucol_d = nc.dram_tensor('ucol', [128, KT], F32, kind='ExternalInput')
    wout2b_d = nc.dram_tensor('wout2b', [128, KT, MODEL_DIM], BF16, kind='ExternalInput')
    boutb_d = nc.dram_tensor('boutb', [128, MODEL_DIM], F32, kind='ExternalInput')
    ident_d = nc.dram_tensor('ident', [128, 128], BF16, kind='ExternalInput')
    ob_d = nc.dram_tensor('ob', [L, MODEL_DIM], F32, kind='ExternalOutput')
    if debug:
        dbg = {
            'dbg_w1qk': nc.dram_tensor('dbg_w1qk', [128, KT, KT, 256], BF16,
                                       kind='ExternalOutput'),
            'dbg_zT': nc.dram_tensor('dbg_zT', [128, KT, L], BF16,
                                     kind='ExternalOutput'),
            'dbg_v': nc.dram_tensor('dbg_v', [128, LT, NUM_HEADS * 65], BF16,
                                    kind='ExternalOutput'),
            'dbg_o2': nc.dram_tensor('dbg_o2', [128, KT, L], BF16,
                                     kind='ExternalOutput'),
            'dbg_pvs': nc.dram_tensor('dbg_pvs', [1, 512], F32,
                                      kind='ExternalOutput'),
            'dbg_rc': nc.dram_tensor('dbg_rc', [1, 512], F32,
                                     kind='ExternalOutput'),
            'dbg_rcb': nc.dram_tensor('dbg_rcb', [64, 512], F32,
                                      kind='ExternalOutput'),
            'dbg_qt': nc.dram_tensor('dbg_qt', [128, L], BF16,
                                     kind='ExternalOutput'),
            'dbg_kp': nc.dram_tensor('dbg_kp', [128, L], BF16,
                                     kind='ExternalOutput'),
            'dbg_posT': nc.dram_tensor('dbg_posT', [128, KT, L], BF16,
                                       kind='ExternalOutput'),
            'dbg_cb': nc.dram_tensor('dbg_cb', [128, LT, NUM_HEADS], F32,
                                     kind='ExternalOutput'),
        }

    with nc.allow_low_precision(reason="bf16 matmul pipeline"), \
            tile.TileContext(nc) as tc:
        with (
            tc.tile_pool(name='const', bufs=1) as constp,
            tc.tile_pool(name='acts', bufs=1) as acts,
            tc.tile_pool(name='wts', bufs=1) as wts,
            tc.tile_pool(name='xz', bufs=4) as xzp,
            tc.tile_pool(name='xtp', bufs=8) as xtp,
            tc.tile_pool(name='qkp', bufs=2) as qkp,
            tc.tile_pool(name='pt', bufs=4) as ptp,
            tc.tile_pool(name='rc', bufs=4) as rcp,
            tc.tile_pool(name='outp', bufs=3) as outp,
            tc.tile_pool(name='psM', bufs=2, space='PSUM') as psM,
        ):
            # ---------------- constants / weights ----------------
            cb_sb = constp.tile([128, LT, NUM_HEADS], F32)
            ucol_sb = constp.tile([128, KT], F32)
            ident_sb = constp.tile([128, 128], BF16)
            nc.gpsimd.dma_start(ident_sb[:], ident_d[:])
            eps_sb = constp.tile([128, 1], F32)
            nc.vector.memset(eps_sb[:], EPS)

            # startup is DMA-latency-bound: all three hwdge queues
            # (sync/scalar/gpsimd) carry x tiles first (program order per
            # queue), then weight blocks ordered by first-use time; nothing
            # is gated, HBM saturates from ~7us on
            w1v_sb = wts.tile([128, KT, INNER], BF16)
            w1v_r = w1v_d[:].rearrange('(t p) r -> p t r', p=128)
            w1qk_sb = wts.tile([128, KT, KT, 256], BF16)
            posT_sb = wts.tile([128, KT, L], BF16)
            posT_r = posT_d[:].rearrange('(t p) m -> p t m', p=128)
            wout2b_sb = wts.tile([128, KT, MODEL_DIM], BF16)
            boutb_sb = constp.tile([128, MODEL_DIM], F32)

            zT = acts.tile([128, KT, L], BF16)
            # V with a ones column per head: [m-part, mt, h*65+c]
            v_sb = acts.tile([128, LT, NUM_HEADS * 65], BF16)
            nc.vector.memset(
                v_sb[:].rearrange('p t (h c) -> p t h c', c=65)[:, :, :, 64:65], 1.0)
            # normalized attention output, head pairs packed: [128, pair, L]
            outT2 = acts.tile([128, KT, L], BF16)

            # ---------------- LayerNorm + transpose into zT ----------------
            # software-pipelined by one stage: stats/aggr of tile lt are
            # emitted before the normalize/transpose of tile lt-1 so the
            # in-order DVE queue reaches zt(0) as soon as its rstd is ready
            ln_state = {}

            def ln_front(lt):
                xt = xtp.tile([128, MODEL_DIM], F32, tag='xt', name=f'xt{lt}')
                # x round-robins all three hwdge queues; x0 is further split
                # across two rings (its landing gates everything)
                if lt == 0:
                    nc.sync.dma_start(xt[0:64, :], xb_d[0:64, :])
                    nc.scalar.dma_start(xt[64:128, :], xb_d[64:128, :])
                else:
                    eng = (None, nc.gpsimd, nc.sync, nc.scalar,
                           nc.gpsimd, nc.sync, nc.scalar, nc.gpsimd)[lt]
                    eng.dma_start(xt[:], xb_d[lt * 128:(lt + 1) * 128, :])
                stats = xzp.tile([128, 6], F32, tag='stats')
                nc.vector.bn_stats(stats[:], xt[:])
                mv = xzp.tile([128, 2], F32, tag='mv')
                nc.vector.bn_aggr(mv[:], stats[:])
                lnv = xzp.tile([128, 1], F32, tag='lnv')
                nc.scalar.activation(lnv[:], mv[:, 1:2], AF.Ln, bias=eps_sb[:], scale=1.0)
                rstd = xzp.tile([128, 1], F32, tag='rstd', name=f'rstd{lt}')
                nc.scalar.activation(rstd[:], lnv[:], AF.Exp, scale=-0.5)
                ln_state[lt] = (xt, mv, rstd)

            def ln_back(lt):
                xt, mv, rstd = ln_state.pop(lt)
                nmr = xzp.tile([128, 1], F32, tag='nmr')
                nc.vector.scalar_tensor_tensor(nmr[:], mv[:, 0:1], -1.0, rstd[:],
                                               op0=ALU.mult, op1=ALU.mult)
                zt = xzp.tile([128, MODEL_DIM], BF16, tag='zt')
                nc.vector.tensor_scalar(zt[:], xt[:], rstd[:], nmr[:],
                                        op0=ALU.mult, op1=ALU.add)
                tp = psM.tile([128, L], F32, tag='S')
                tpb = tp.bitcast(BF16)
                for c in range(KT):
                    nc.tensor.transpose(tpb[:, c * 128:(c + 1) * 128],
                                        zt[:, c * 128:(c + 1) * 128], ident_sb[:])
                nc.vector.tensor_copy(
                    zT[:, :, lt * 128:(lt + 1) * 128],
                    tpb[:, 0:512].rearrange('p (c l) -> p c l', c=KT))

            for lt in range(LT):
                ln_front(lt)
                if lt >= 1:
                    ln_back(lt - 1)
            ln_back(LT - 1)

            # weight DMAs: emitted after the LN loop so each queue serves its
            # x tiles first, then weights in first-use order. sync is idle
            # all kernel -> it carries the early-needed qk/pos blocks.
            nc.sync.dma_start(w1qk_sb[:, 0], w1qk_d[:, 0])
            nc.sync.dma_start(posT_sb[:, 0, :], posT_r[:, 0, :])
            nc.sync.dma_start(w1qk_sb[:, 1], w1qk_d[:, 1])
            nc.sync.dma_start(posT_sb[:, 2, :], posT_r[:, 2, :])
            nc.sync.dma_start(posT_sb[:, 3, :], posT_r[:, 3, :])
            nc.gpsimd.dma_start(cb_sb[:], cb_d[:])
            nc.gpsimd.dma_start(ucol_sb[:], ucol_d[:])
            nc.gpsimd.dma_start(w1v_sb[:], w1v_r)
            nc.gpsimd.dma_start(w1qk_sb[:, 2], w1qk_d[:, 2])
            nc.gpsimd.dma_start(w1qk_sb[:, 3], w1qk_d[:, 3])
            nc.gpsimd.dma_start(posT_sb[:, 1, :], posT_r[:, 1, :])
            nc.gpsimd.dma_start(wout2b_sb[:], wout2b_d[:])
            nc.gpsimd.dma_start(boutb_sb[:], boutb_d[:])


            # ---------------- interleaved projections + attention ----------
            qts, kps = {}, {}

            def qk_chains(t):
                """allocate qt/kp for pair t and return the 4 projection
                chains as closures (woven into the previous pair's scores
                loop as PE fillers; they depend only on zT and w1qk)"""
                qt = qkp.tile([128, L], BF16, tag='qT', name=f'qT{t}')
                kp = qkp.tile([128, L], BF16, tag='kT', name=f'kT{t}')
                qts[t], kps[t] = qt, kp
                chains = []
                for which, dst in ((1, kp), (0, qt)):
                    for lc in range(2):
                        def chain(which=which, dst=dst, lc=lc):
                            ws = slice(which * 128, (which + 1) * 128)
                            ls = slice(lc * 512, (lc + 1) * 512)
                            acc = psM.tile([128, 512], F32, tag='qkv',
                                           name=f'qk{t}_{which}_{lc}')
                            for kt in range(KT):
                                nc.tensor.matmul(
                                    acc[:], w1qk_sb[:, t, kt, ws],
                                    zT[:, kt, ls],
                                    start=(kt == 0), stop=(kt == KT - 1))
                            if which == 0:
                                # q' = q + u_h  (per-partition add)
                                nc.vector.tensor_scalar_add(dst[:, ls], acc[:],
                                                            ucol_sb[:, t:t + 1])
                            else:
                                # kp = k + pos
                                nc.vector.tensor_tensor(dst[:, ls], acc[:],
                                                        posT_sb[:, t, ls], op=ALU.add)
                        chains.append(chain)
                return chains

            def emit_scores(t, fillers=()):
                """Scores + exp for heads 2t, 2t+1, interleaved at mt grain so
                the two heads' (64,128) stationary tiles ping-pong quadrants.
                The scores loop is ACT-paced (2 exps per mt vs 4 small score
                matmuls), so one contiguous filler chain (a v-projection or a
                previous pair's PV chain) is emitted per mt iteration to keep
                the PE dense while ACT chews."""
                qt, kp = qts[t], kps[t]
                fillers = list(fillers)
                PTs = {}
                for h in (2 * t, 2 * t + 1):
                    PTs[h] = ptp.tile([128, LT, L], BF16, tag='PT', name=f'PT{h}')
                for mt in range(LT):
                    for h in (2 * t, 2 * t + 1):
                        hp = 64 * (h % 2)
                        prow = slice(hp, hp + 64)
                        sacc = psM.tile([128, L], F32, tag='S')
                        for lc in range(2):
                            ls = slice(lc * 512, (lc + 1) * 512)
                            nc.tensor.matmul(sacc[:, ls],
                                             kp[prow, mt * 128:(mt + 1) * 128],
                                             qt[prow, ls], start=True, stop=True)
                        nc.scalar.activation(PTs[h][:, mt, :], sacc[:], AF.Exp,
                                             bias=cb_sb[:, mt, h:h + 1], scale=SCALE)
                    if fillers:
                        fillers.pop(0)()
                while fillers:
                    fillers.pop(0)()
                return PTs

            def pv_step(pvacc, h, PT, lc, mt):
                ls = slice(lc * 512, (lc + 1) * 512)
                nc.tensor.matmul(pvacc[:], v_sb[:, mt, h * 65:(h + 1) * 65],
                                 PT[:, mt, ls],
                                 start=(mt == 0), stop=(mt == LT - 1))

            def pv_finish(pvacc, h, lc):
                """normalize the finished P^T @ [V|1] into packed outT2"""
                pair, hp = divmod(h, 2)
                rows = slice(hp * 64, hp * 64 + 64)
                ls = slice(lc * 512, (lc + 1) * 512)
                zrow = rcp.tile([1, 512], F32, tag='zrow')
                nc.vector.tensor_copy(zrow[:], pvacc[64:65, :])
                rc = rcp.tile([1, 512], F32, tag='rc')
                # custom-DVE op: partition-0 SBUF source only (PSUM reads and
                # partition-offset sources both misparse on HW)
                nc.vector.reciprocal_approx_fast(rc[:], zrow[:])
                rcb = rcp.tile([64, 512], F32, tag='rcb')
                nc.gpsimd.partition_broadcast(rcb[:], rc[:])
                nc.vector.tensor_mul(outT2[rows, pair, ls], pvacc[0:64, :], rcb[:])
                fin_state['last'] = (zrow, rc, rcb)

            fin_state = {}

            def emit_pv_block(hs, PTs_, lc):
                """contiguous PV chains for the given heads at one lc.
                lc1 borrows the S ring so all 4 end chains hold distinct
                PSUM slots and none WAR-stalls on another's division."""
                for h in hs:
                    pv_chain(h, PTs_[h], lc, end=(lc == 1))

            def pv_chain(h, PT, lc, end=False):
                if end:
                    # the last pair's 4 chains borrow the S ring (idle after
                    # the final exps) so none of them WAR-stalls on a previous
                    # chain's division (copy/recip/bcast/mul) still reading
                    # its pv-ring bank
                    big = psM.tile([128, L], F32, tag='S', name=f'pvE{h}_{lc}')
                    pvacc = big[0:65, 0:512]
                else:
                    pvacc = psM.tile([65, 512], F32, tag='pv',
                                     name=f'pv{h}_{lc}')[:]
                for mt in range(LT):
                    pv_step(pvacc, h, PT, lc, mt)
                pv_finish(pvacc, h, lc)

            def v_chain(mt):
                acc = psM.tile([128, 512], F32, tag='qkv', name=f'vacc{mt}')
                for kt in range(KT):
                    nc.tensor.matmul(acc[:], zT[:, kt, mt * 128:(mt + 1) * 128],
                                     w1v_sb[:, kt, :],
                                     start=(kt == 0), stop=(kt == KT - 1))
                nc.vector.tensor_copy(
                    v_sb[:, mt, :].rearrange('p (h c) -> p h c', c=65)[:, :, 0:64],
                    acc[:].rearrange('p (h c) -> p h c', c=64))

            def emit_outproj(lts):
                # the last head pair's contribution is accumulated LAST so the
                # first matmuls of each chain run while that pair's softmax
                # divisions are still in flight on DVE/Pool. b_out is folded
                # into the PSUM->SBUF copy (saves a PE pass per chain); the
                # output DMAs ride the idle sync queue (scalar for the last
                # tile so the final two transfers overlap).
                for lt in lts:
                    facc = psM.tile([128, MODEL_DIM], F32, tag='qkv')
                    for p in range(KT - 1):
                        nc.tensor.matmul(facc[:], outT2[:, p, lt * 128:(lt + 1) * 128],
                                         wout2b_sb[:, p, :],
                                         start=(p == 0), stop=False)
                    nc.tensor.matmul(facc[:], outT2[:, KT - 1, lt * 128:(lt + 1) * 128],
                                     wout2b_sb[:, KT - 1, :],
                                     start=False, stop=True)
                    ot = outp.tile([128, MODEL_DIM], F32, tag='ot')
                    nc.vector.tensor_tensor(ot[:], facc[:], boutb_sb[:], op=ALU.add)
                    eng = nc.scalar if lt == LT - 1 else nc.sync
                    eng.dma_start(ob_d[lt * 128:(lt + 1) * 128, :], ot[:])

            prev = None
            for c in qk_chains(0):
                c()
            for t in range(KT):
                if prev is None:
                    base = [(lambda mt=mt: v_chain(mt)) for mt in range(LT)]
                else:
                    base = [(lambda h=h, lc=lc, PT=prev[h]: pv_chain(h, PT, lc))
                            for lc in (0, 1) for h in prev.keys()]
                nxt = qk_chains(t + 1) if t + 1 < KT else []
                # interleave so the next pair's projections (and their DVE
                # folds) finish well before its scores loop begins
                fillers = []
                for i in range(max(len(base), len(nxt))):
                    if i < len(base):
                        fillers.append(base[i])
                    if i < len(nxt):
                        fillers.append(nxt[i])
                prev = emit_scores(t, fillers)
            # last pair: all PV chains; the lc0 divisions (DVE/Pool latency)
            # hide under the lc1 chains, and the lc1 divisions hide under the
            # first half of the output projection
            phs = list(prev.keys())
            emit_pv_block(phs, prev, 0)
            emit_pv_block(phs, prev, 1)
            emit_outproj(range(0, 4))
            emit_outproj(range(4, 8))

            if debug:
                zrow, rc, rcb = fin_state['last']
                nc.sync.dma_start(dbg['dbg_w1qk'][:], w1qk_sb[:])
                nc.sync.dma_start(dbg['dbg_zT'][:], zT[:])
                nc.sync.dma_start(dbg['dbg_v'][:], v_sb[:])
                nc.sync.dma_start(dbg['dbg_o2'][:], outT2[:])
                nc.sync.dma_start(dbg['dbg_pvs'][:], zrow[:])
                nc.sync.dma_start(dbg['dbg_rc'][:], rc[:])
                nc.sync.dma_start(dbg['dbg_rcb'][:], rcb[:])
                nc.sync.dma_start(dbg['dbg_qt'][:], qts[KT - 1][:])
                nc.sync.dma_start(dbg['dbg_kp'][:], kps[KT - 1][:])
                nc.sync.dma_start(dbg['dbg_posT'][:], posT_sb[:])
                nc.sync.dma_start(dbg['dbg_cb'][:], cb_sb[:])

    # Force all activations (Ln/Exp/Identity) onto the single table set that
    # contains them all — otherwise the table-load picker alternates between
    # the natural_log and exp sets, paying a ~2.7us table load per switch.
    import concourse.bacc as bacc_mod
    orig_tables = bacc_mod.get_activation_tables

    def _only_ln_exp(arch):
        t = orig_tables(arch)
        return {name: (funcs if name == 'natural_log_exp_and_others' else
                       type(funcs)())
                for name, funcs in t.items()}

    bacc_mod.get_activation_tables = _only_ln_exp
    try:
        nc.compile()
    finally:
        bacc_mod.get_activation_tables = orig_tables
    return nc


def _host_prep(x, gamma, beta, w_qkv, b_qkv, w_pos, w_out, b_out, u_bias, v_bias):
    """Host-side layout prep. Returns (common_inputs, per_core_x_list)."""
    import ml_dtypes
    BF = ml_dtypes.bfloat16
    W1 = (gamma[:, None] * w_qkv.T).astype(np.float32)        # [D, 3*INNER]
    b1 = (b_qkv + beta @ w_qkv.T).astype(np.float32)
    if np.any(b1 != 0):
        raise NotImplementedError("nonzero qkv bias not supported by this kernel")
    # pair-major qk weights: [p, pair, kt, 0:128]=q_pair | [..,128:256]=k_pair
    wqk = W1[:, :2 * INNER].reshape(MODEL_DIM, 2, KT, 128)     # [D, q/k, pair, 128]
    wqk = wqk.transpose(0, 2, 1, 3).reshape(MODEL_DIM, KT, 256)
    w1qk = np.ascontiguousarray(
        wqk.reshape(KT, 128, KT, 256).transpose(1, 2, 0, 3)).astype(BF)
    w1v = np.ascontiguousarray(W1[:, 2 * INNER:]).astype(BF)
    # pos = rel_pos_emb @ w_pos^T, computed on host: [L(m), INNER]
    pos = (_rel_pos_emb_np() @ w_pos.T).astype(np.float32)
    posT = np.ascontiguousarray(pos.T).astype(BF)              # [INNER, L]
    # exp bias table c_h[m] = SCALE * (v_h - u_h) . pos_h[m], m-major tiles
    dvu = (v_bias - u_bias).astype(np.float32)                 # [H, Dh]
    cb = np.einsum('mhd,hd->mh', pos.reshape(L, NUM_HEADS, DIM_HEAD), dvu)
    cb = cb * SCALE                                            # [L(m), H]
    cb = cb.reshape(LT, 128, NUM_HEADS).transpose(1, 0, 2)
    cb = np.ascontiguousarray(cb).astype(np.float32)           # [128, LT, H]
    # u columns: ucol[:, t] = [u_{2t} | u_{2t+1}]
    ucol = np.ascontiguousarray(
        u_bias.reshape(KT, 128).T).astype(np.float32)          # [128, KT]
    # out projection, head pairs packed along K
    wout2b = np.ascontiguousarray(
        w_out.T.reshape(KT, 128, MODEL_DIM).transpose(1, 0, 2)).astype(BF)
    common = {
        'w1qk': w1qk, 'w1v': w1v, 'posT': posT, 'cb': cb, 'ucol': ucol,
        'wout2b': wout2b,
        'boutb': np.ascontiguousarray(
            np.broadcast_to(b_out.astype(np.float32), (128, MODEL_DIM))),
        'ident': np.eye(128).astype(BF),
    }
    xs = [np.ascontiguousarray(x[b]) for b in range(N_CORES)]
    return common, xs


def kernel(x, gamma, beta, w_qkv, b_qkv, w_pos, w_out, b_out, u_bias, v_bias):
    x = np.asarray(x, np.float32)
    args = [np.asarray(a, np.float32) for a in
            (gamma, beta, w_qkv, b_qkv, w_pos, w_out, b_out, u_bias, v_bias)]
    common, xs = _host_prep(x, *args)

    if 'nc' not in _CACHE:
        _CACHE['nc'] = _build()
    nc = _CACHE['nc']

    from concourse.bass_utils import run_bass_kernel_spmd
    in_maps = [{'xb': xs[b], **common} for b in range(N_CORES)]
    res = run_bass_kernel_spmd(nc, in_maps, core_ids=list(range(N_CORES)))
    return np.stack([res.results[b]['ob'] for b in range(N_CORES)], axis=0)

